# revision 20
# baseline (speedup 1.0000x reference)
"""Distributed Trainium2 kernel for nn_Attention (RMSNorm + QKV + RoPE +
causal SDPA + out-proj), Megatron-style head-parallel over 8 NeuronCores.

Strategy per core c (owns heads 2c, 2c+1):
  phase 0: RMSNorm of its own 512-token chunk, in transposed layout
           (features on partitions), via ones-matmul column reduction.
  AG:      AllGather of normalized activations xn^T (bf16, 1MB/rank).
  phase 3: Q/K/V projections for its two heads (bf16 matmuls), RoPE via
           host-precomputed cos/sin tables + partition-swap, V transposed
           on TensorE into [keys, dims] blocks with a ones column appended
           (softmax denominator comes out of the same matmul as context).
  phase 4: causal SDPA in S^T layout: scores^T = K^T_blk.T @ Q^T
           (two heads row-packed on the PE array), exp on ScalarE (no max
           subtraction - scores are O(6) so fp32 exp is safe), diag-block
           masking, AV matmul with ones-augmented V accumulating
           [ctx^T; denom] in PSUM; deferred division by the denominator.
  A2A:     AllToAll redistributes context from head-sharded to
           token-sharded (1MB/rank).
  phase 6: out-projection for its own 512-token chunk only.
Host does layout-only prep (transpose, head-column permutation, constant
RoPE/mask tables) and the final concat.
"""
import sys

sys.path.insert(0, "/opt/trn_rl_repo")

import numpy as np
import ml_dtypes
from contextlib import ExitStack

import concourse.bass as bass
import concourse.mybir as mybir
import concourse.tile as tile
from concourse import bacc
from concourse.bass_utils import run_bass_kernel_spmd
from concourse.masks import make_identity

F32 = mybir.dt.float32
BF16 = mybir.dt.bfloat16

B, S, D, H, DH = 2, 2048, 1024, 16, 64
NC = 8
TOK = B * S            # 4096
CHUNK = TOK // NC      # 512
EPS = 1.1920929e-07
THETA = 10000.0
NKB = S // 128         # key blocks per batch: 16
QT = S // 512          # q tiles per batch: 4

_CACHE = {}
DEBUG = False


def _build():
    nc = bacc.Bacc("TRN2", target_bir_lowering=False, debug=False, num_devices=NC)

    xt_d = nc.dram_tensor("xt", [D, CHUNK], F32, kind="ExternalInput")
    nw_d = nc.dram_tensor("nw", [D, 1], F32, kind="ExternalInput")
    wq_d = nc.dram_tensor("wqc", [D, 128], F32, kind="ExternalInput")
    wk_d = nc.dram_tensor("wkc", [D, 128], F32, kind="ExternalInput")
    wv_d = nc.dram_tensor("wvc", [D, 128], F32, kind="ExternalInput")
    wo_d = nc.dram_tensor("wo", [D, D], F32, kind="ExternalInput")
    cos_d = nc.dram_tensor("cosb", [128, TOK], BF16, kind="ExternalInput")
    sin_d = nc.dram_tensor("sinb", [128, TOK], BF16, kind="ExternalInput")
    msk_d = nc.dram_tensor("dmask", [128, 128], BF16, kind="ExternalInput")
    out_d = nc.dram_tensor("out", [CHUNK, D], F32, kind="ExternalOutput")

    agx_in = nc.dram_tensor("agx_in", [D, CHUNK], BF16)
    agx_out = nc.dram_tensor("agx_out", [NC * D, CHUNK], BF16, addr_space="Shared")
    agr_in = nc.dram_tensor("agr_in", [1, CHUNK], BF16)
    agr_out = nc.dram_tensor("agr_out", [NC, CHUNK], BF16, addr_space="Shared")
    a2a_in = nc.dram_tensor("a2a_in", [NC * 128, CHUNK], BF16)
    a2a_out = nc.dram_tensor("a2a_out", [NC * 128, CHUNK], BF16)
    dbg = {}
    if DEBUG:
        for nm, shp in (("dbg_xn0", [128, TOK]), ("dbg_q", [128, TOK]),
                        ("dbg_k", [128, TOK]), ("dbg_v", [128, TOK]),
                        ("dbg_vall", [128, B * NKB * 130]),
                        ("dbg_ctx", [128, TOK]), ("dbg_ctxg", [128, NC * CHUNK])):
            dbg[nm] = nc.dram_tensor(nm, shp, BF16, kind="ExternalOutput")
        for nm in ("dbg_den", "dbg_rec"):
            dbg[nm] = nc.dram_tensor(nm, [16, 512], F32, kind="ExternalOutput")

    with tile.TileContext(nc) as tc, ExitStack() as ctx:
        pp = ctx.enter_context(tc.tile_pool(name="persist", bufs=1))

        # ---- persistent tiles ----
        qT = pp.tile([128, TOK], BF16, tag="qT")
        kT = pp.tile([128, TOK], BF16, tag="kT")
        vT = pp.tile([128, TOK], BF16, tag="vT")
        v_all = pp.tile([128, B * NKB * 130], BF16, tag="v_all")
        cosT = pp.tile([128, TOK], BF16, tag="cosT")
        sinT = pp.tile([128, TOK], BF16, tag="sinT")
        dmaskT = pp.tile([128, 128], BF16, tag="dmaskT")
        identT = pp.tile([128, 128], BF16, tag="identT")
        ones128 = pp.tile([128, 1], BF16, tag="ones128")
        ones1 = pp.tile([1, 128], BF16, tag="ones1")
        nw_sb = pp.tile([128, 8], F32, tag="nw_sb")
        w_sb = {p: pp.tile([128, 8, 128], BF16, name=f"w_{p}", tag=f"w_{p}") for p in "qkv"}
        wo_sb = pp.tile([128, 8, 1024], BF16, tag="wo_sb")
        ctx_sb = pp.tile([128, TOK], BF16, tag="ctx_sb")

        nc.vector.memset(ones128, 1.0)
        nc.vector.memset(ones1, 1.0)

        # ---- phase 0: cast raw x^T chunk to bf16 + own-chunk inv-rms ----
        # The AllGather ships RAW x (bf16) plus a per-chunk 1/rms row; the
        # normalization is applied after the gather by folding 1/rms into the
        # RoPE tables (RoPE commutes with per-token scalars) and one extra
        # multiply for V. norm_w is folded into the weight casts. This gets
        # the collective started ~60us earlier than normalizing first.
        with tc.tile_pool(name="rms", bufs=2) as rms_pool, \
             tc.tile_pool(name="ps0", bufs=1, space="PSUM") as ps0:
            ssq = ps0.tile([1, CHUNK], F32, tag="ssq")
            for kt in range(8):
                xb = rms_pool.tile([128, CHUNK], BF16, tag="xb")
                for h in range(2):
                    r0 = kt * 128 + h * 64
                    xtile = rms_pool.tile([64, CHUNK], F32, tag=f"xt{h}")
                    nc.sync.dma_start(out=xtile, in_=xt_d[r0 : r0 + 64, :])
                    nc.vector.tensor_copy(xb[h * 64 : (h + 1) * 64, :], xtile)
                    nc.sync.dma_start(out=agx_in[r0 : r0 + 64, :], in_=xb[h * 64 : (h + 1) * 64, :])
                xsq = rms_pool.tile([128, CHUNK], BF16, tag="xsq")
                nc.vector.tensor_mul(xsq, xb, xb)
                nc.tensor.matmul(ssq, ones128, xsq, start=(kt == 0), stop=(kt == 7))
            eps_t = rms_pool.tile([1, 1], F32, tag="eps_t")
            nc.vector.memset(eps_t, float(EPS))
            rstd = rms_pool.tile([1, CHUNK], F32, tag="rstd")
            nc.scalar.activation(rstd, ssq, mybir.ActivationFunctionType.Sqrt,
                                 bias=eps_t[0:1, 0:1], scale=1.0 / D)
            inv = rms_pool.tile([1, CHUNK], F32, tag="inv")
            nc.vector.reciprocal_approx_fast(out=inv, in_=rstd)
            invb = rms_pool.tile([1, CHUNK], BF16, tag="invb")
            nc.vector.tensor_copy(invb, inv)
            nc.sync.dma_start(out=agr_in[0:1, :], in_=invb)

        # ---- AllGather raw x^T + inv-rms rows ----
        nc.gpsimd.collective_compute(
            "AllGather", mybir.AluOpType.bypass,
            replica_groups=[list(range(NC))],
            ins=[agx_in.ap().opt()], outs=[agx_out.ap().opt()])
        nc.gpsimd.collective_compute(
            "AllGather", mybir.AluOpType.bypass,
            replica_groups=[list(range(NC))],
            ins=[agr_in.ap().opt()], outs=[agr_out.ap().opt()])

        nc.sync.dma_start(out=cosT, in_=cos_d[:, :])
        nc.sync.dma_start(out=sinT, in_=sin_d[:, :])
        nc.sync.dma_start(out=dmaskT, in_=msk_d[:, :])
        make_identity(nc, identT)
        for kt in range(8):
            nc.sync.dma_start(out=nw_sb[:, kt : kt + 1], in_=nw_d[kt * 128 : (kt + 1) * 128, :])

        # weight staging + cast to bf16
        with tc.tile_pool(name="wstage", bufs=3) as wstage:
            for p, d in (("q", wq_d), ("k", wk_d), ("v", wv_d)):
                for kt in range(8):
                    st = wstage.tile([128, 128], F32, tag="wst")
                    nc.sync.dma_start(out=st, in_=d[kt * 128 : (kt + 1) * 128, :])
                    nc.vector.tensor_scalar_mul(w_sb[p][:, kt, :], st, nw_sb[:, kt : kt + 1])


        with tc.tile_pool(name="xn", bufs=1) as xn_pool, \
             tc.tile_pool(name="tmp", bufs=1) as tmp_pool, \
             tc.tile_pool(name="ps3", bufs=3, space="PSUM") as ps3:
            # gathered raw x^T -> SBUF as per-(ktile, chunk) tiles so each
            # projection matmul only waits for its own 128KB slab
            agx_v = agx_out.ap().rearrange("(c r) t -> r c t", r=D)
            xn_sb = [[None] * NC for _ in range(8)]
            for kt in range(8):
                for c in range(NC):
                    t = xn_pool.tile([128, CHUNK], BF16, name=f"xn{kt}_{c}", tag=f"xn{kt}_{c}")
                    nc.sync.dma_start(out=t, in_=agx_v[kt * 128 : (kt + 1) * 128, c])
                    xn_sb[kt][c] = t
            # inv-rms rows -> [1, 4096], broadcast to rbc [128, 4096]
            invg = xn_pool.tile([1, TOK], BF16, tag="invg")
            nc.sync.dma_start(out=invg.rearrange("p (c t) -> p c t", c=NC),
                              in_=agr_out.ap()[:, :])
            rbc = xn_pool.tile([128, TOK], BF16, tag="rbc")
            for tt in range(8):
                rp = ps3.tile([128, 512], F32, name="rp", tag="rp", bufs=2)
                nc.tensor.matmul(rp, ones1, invg[0:1, tt * 512 : (tt + 1) * 512],
                                 start=True, stop=True)
                nc.scalar.copy(rbc[:, tt * 512 : (tt + 1) * 512], rp)
            # fold 1/rms into the RoPE tables (q, k) ...
            nc.vector.tensor_mul(cosT, cosT, rbc)
            nc.vector.tensor_mul(sinT, sinT, rbc)

            # ---- phase 3: projections + RoPE + V transpose ----
            def project(dest, wkey):
                for tt in range(8):
                    ps = ps3.tile([128, 512], F32, tag="proj")
                    for kt in range(8):
                        nc.tensor.matmul(ps, w_sb[wkey][:, kt, :],
                                         xn_sb[kt][tt],
                                         start=(kt == 0), stop=(kt == 7))
                    nc.scalar.copy(dest[:, tt * 512 : (tt + 1) * 512], ps)

            def rope(t):
                sw = tmp_pool.tile([128, TOK], BF16, tag="sw")
                for a, b2 in ((0, 32), (64, 96)):
                    nc.sync.dma_start(out=sw[a : a + 32, :], in_=t[b2 : b2 + 32, :])
                    nc.sync.dma_start(out=sw[b2 : b2 + 32, :], in_=t[a : a + 32, :])
                nc.vector.tensor_mul(t, t, cosT)
                nc.vector.tensor_mul(sw, sw, sinT)
                nc.vector.tensor_add(t, t, sw)

            if DEBUG:
                nc.sync.dma_start(out=dbg["dbg_xn0"][:, :], in_=xn_sb[0])
            project(qT, "q")
            rope(qT)
            project(kT, "k")
            rope(kT)
            project(vT, "v")
            nc.vector.tensor_mul(vT, vT, rbc)
            vv = v_all.rearrange("p (blk c) -> p blk c", c=130)
            for b in range(B):
                for kb in range(NKB):
                    blk = b * NKB + kb
                    pst = ps3.tile([128, 128], BF16, tag="vtr")
                    nc.tensor.transpose(pst, vT[:, b * S + kb * 128 : b * S + (kb + 1) * 128], identT)
                    nc.scalar.copy(vv[:, blk, 0:64], pst[:, 0:64])
                    nc.scalar.copy(vv[:, blk, 65:129], pst[:, 64:128])
            nc.gpsimd.memset(vv[:, :, 64:65], 1.0)
            nc.gpsimd.memset(vv[:, :, 129:130], 1.0)
            if DEBUG:
                nc.sync.dma_start(out=dbg["dbg_q"][:, :], in_=qT)
                nc.sync.dma_start(out=dbg["dbg_k"][:, :], in_=kT)
                nc.sync.dma_start(out=dbg["dbg_v"][:, :], in_=vT)
                nc.sync.dma_start(out=dbg["dbg_vall"][:, :], in_=v_all)

        # stage wo during SDPA (emitted late so its DMAs don't contend with
        # the front of the kernel)
        with tc.tile_pool(name="wstage2", bufs=2) as wstage2:
            for kt in range(8):
                st = wstage2.tile([128, 1024], F32, tag="wost")
                nc.sync.dma_start(out=st, in_=wo_d[kt * 128 : (kt + 1) * 128, :])
                nc.vector.tensor_copy(wo_sb[:, kt, :], st)

        # ---- phase 4: SDPA ----
        # The two batches are interleaved so one batch's exp (ScalarE) overlaps
        # the other's matmuls (TensorE). Scores share one 3-buf PSUM tag; the
        # four context accumulators (2 batches x 2 heads) get their own banks.
        with tc.tile_pool(name="pexp", bufs=4) as pexp, \
             tc.tile_pool(name="cnorm", bufs=2) as cnorm, \
             tc.tile_pool(name="ps4", bufs=3, space="PSUM") as ps4, \
             tc.tile_pool(name="ps4b", bufs=1, space="PSUM") as ps4b, \
             tc.tile_pool(name="ps4c", bufs=1, space="PSUM") as ps4c:
            for step in range(B * QT):
                b, j = step % B, step // B
                base = b * S
                ctxp = {0: ps4c.tile([65, 512], F32, name=f"ctxA{b}", tag=f"ctxA{b}"),
                        1: ps4c.tile([65, 512], F32, name=f"ctxB{b}", tag=f"ctxB{b}")}
                nkb = 4 * (j + 1)
                for kb in range(nkb):
                    m = kb - 4 * j
                    c0 = 128 * m if m >= 0 else 0
                    w = 512 - c0
                    qcol0 = base + 512 * j + c0
                    koff = base + kb * 128
                    for hi, r0 in ((0, 0), (1, 64)):
                        sc = ps4.tile([128, 512], F32, name="sc", tag="sc")
                        nc.tensor.matmul(
                            sc[:, 0:w],
                            kT[r0 : r0 + 64, koff : koff + 128],
                            qT[r0 : r0 + 64, qcol0 : qcol0 + w],
                            start=True, stop=True)
                        p = pexp.tile([128, 512], BF16, name=f"p{hi}", tag=f"p{hi}")
                        nc.scalar.activation(
                            p[:, c0:512], sc[:, 0:w],
                            mybir.ActivationFunctionType.Exp, scale=0.125)
                        if m >= 0:
                            nc.vector.tensor_mul(p[:, c0 : c0 + 128], p[:, c0 : c0 + 128], dmaskT)
                        vcol = (b * NKB + kb) * 130 + hi * 65
                        nc.tensor.matmul(
                            ctxp[hi][:, c0:512],
                            v_all[:, vcol : vcol + 65],
                            p[:, c0:512],
                            start=(kb == 0), stop=(kb == nkb - 1),
                            skip_group_check=True)
                # normalize: ctx / denom (denom = row 64 of ctx psum)
                for hi, r0 in ((0, 0), (1, 64)):
                    den_s = cnorm.tile([1, 512], F32, tag="den_s")
                    nc.vector.tensor_copy(den_s, ctxp[hi][64:65, :])
                    rec = cnorm.tile([1, 512], F32, tag="rec")
                    nc.vector.reciprocal_approx_fast(out=rec, in_=den_s)
                    if DEBUG:
                        di = ((b * QT + j) * 2 + hi)
                        nc.sync.dma_start(out=dbg["dbg_den"][di : di + 1, :], in_=den_s)
                        nc.sync.dma_start(out=dbg["dbg_rec"][di : di + 1, :], in_=rec)
                    recb = cnorm.tile([1, 512], BF16, tag="recb")
                    nc.vector.tensor_copy(recb, rec)
                    bc = ps4b.tile([64, 512], F32, name="bc", tag="bc")
                    nc.tensor.matmul(bc, ones1[0:1, 0:64], recb, start=True, stop=True)
                    cth = cnorm.tile([64, 512], F32, tag="cth")
                    nc.vector.tensor_copy(cth, ctxp[hi][0:64, :])
                    nc.vector.tensor_mul(
                        ctx_sb[r0 : r0 + 64, base + 512 * j : base + 512 * (j + 1)],
                        cth, bc)

        if DEBUG:
            nc.sync.dma_start(out=dbg["dbg_ctx"][:, :], in_=ctx_sb)
        # ---- AllToAll: head-sharded ctx -> token-sharded ctx ----
        # a2a_in block c' = our ctx rows for chunk c'; received block i =
        # rank i's ctx rows (global rows 128i..128i+127) for OUR chunk.
        for cb in range(NC):
            nc.sync.dma_start(out=a2a_in[cb * 128 : (cb + 1) * 128, :],
                              in_=ctx_sb[:, cb * CHUNK : (cb + 1) * CHUNK])
        nc.gpsimd.collective_compute(
            "AllToAll", mybir.AluOpType.bypass,
            replica_groups=[list(range(NC))],
            ins=[a2a_in.ap().opt()], outs=[a2a_out.ap().opt()])

        # ---- phase 6: out-projection on own token chunk ----
        with tc.tile_pool(name="ctxgp", bufs=1) as ctxgp, \
             tc.tile_pool(name="outp", bufs=3) as outp, \
             tc.tile_pool(name="ps6", bufs=2, space="PSUM") as ps6:
            ctxg = ctxgp.tile([128, NC * CHUNK], BF16, tag="ctxg")
            for cb in range(NC):
                nc.sync.dma_start(out=ctxg[:, cb * CHUNK : (cb + 1) * CHUNK],
                                  in_=a2a_out[cb * 128 : (cb + 1) * 128, :])
            if DEBUG:
                nc.sync.dma_start(out=dbg["dbg_ctxg"][:, :], in_=ctxg)
            for tl in range(4):
                pso = {nh: ps6.tile([128, 512], F32, name=f"op{nh}", tag=f"op{nh}") for nh in range(2)}
                for nh in range(2):
                    for cb in range(8):
                        nc.tensor.matmul(
                            pso[nh],
                            ctxg[:, cb * CHUNK + tl * 128 : cb * CHUNK + (tl + 1) * 128],
                            wo_sb[:, cb, nh * 512 : (nh + 1) * 512],
                            start=(cb == 0), stop=(cb == 7))
                ost = outp.tile([128, 1024], F32, tag="ost")
                nc.scalar.copy(ost[:, 0:512], pso[0])
                nc.scalar.copy(ost[:, 512:1024], pso[1])
                nc.sync.dma_start(out=out_d[tl * 128 : (tl + 1) * 128, :], in_=ost)

    nc.compile()
    return nc


def _head_cols(h, deinterleave):
    base = h * DH
    if deinterleave:
        return np.concatenate([base + np.arange(0, DH, 2), base + np.arange(1, DH, 2)])
    return base + np.arange(DH)


def _make_tables():
    inv_freq = 1.0 / (THETA ** (np.arange(0, DH, 2) / DH))   # [32]
    ang = np.arange(S)[:, None] * inv_freq[None, :]          # [2048, 32]
    ch = np.cos(ang).T.astype(np.float32)                    # [32, 2048]
    sh = np.sin(ang).T.astype(np.float32)
    cosb = np.tile(np.concatenate([ch, ch, ch, ch], axis=0), (1, B))
    sinb = np.tile(np.concatenate([-sh, sh, -sh, sh], axis=0), (1, B))
    kk, qq = np.meshgrid(np.arange(128), np.arange(128), indexing="ij")
    dmask = (kk <= qq).astype(np.float32)
    bf = ml_dtypes.bfloat16
    return cosb.astype(bf), sinb.astype(bf), dmask.astype(bf)


def _in_maps(inputs):
    x = np.ascontiguousarray(inputs["x"], dtype=np.float32)
    norm_w = np.asarray(inputs["norm_w"], dtype=np.float32)
    wq = np.asarray(inputs["wq"], dtype=np.float32)
    wk = np.asarray(inputs["wk"], dtype=np.float32)
    wv = np.asarray(inputs["wv"], dtype=np.float32)
    wo = np.ascontiguousarray(inputs["wo"], dtype=np.float32)

    xT = np.ascontiguousarray(x.reshape(TOK, D).T)           # [1024, 4096]
    cosb, sinb, dmask = _make_tables()
    nw = np.ascontiguousarray(norm_w.reshape(D, 1))

    maps = []
    for c in range(NC):
        hA, hB = 2 * c, 2 * c + 1
        qcols = np.concatenate([_head_cols(hA, True), _head_cols(hB, True)])
        vcols = np.concatenate([_head_cols(hA, False), _head_cols(hB, False)])
        maps.append({
            "xt": np.ascontiguousarray(xT[:, c * CHUNK : (c + 1) * CHUNK]),
            "nw": nw,
            "wqc": np.ascontiguousarray(wq[:, qcols]),
            "wkc": np.ascontiguousarray(wk[:, qcols]),
            "wvc": np.ascontiguousarray(wv[:, vcols]),
            "wo": wo,
            "cosb": cosb,
            "sinb": sinb,
            "dmask": dmask,
        })
    return maps


def _run(inputs, trace=False):
    if "nc" not in _CACHE:
        _CACHE["nc"] = _build()
    nc = _CACHE["nc"]
    res = run_bass_kernel_spmd(nc, _in_maps(inputs), core_ids=list(range(NC)),
                               trace=trace)
    chunks = [res.results[c]["out"] for c in range(NC)]
    out = np.concatenate(chunks, axis=0).reshape(B, S, D).astype(np.float32)
    return out, res


def kernel(**inputs) -> np.ndarray:
    out, _ = _run(inputs, trace=False)
    return out


# revision 21
# speedup vs baseline: 1.0030x; 1.0030x over previous
"""Distributed Trainium2 kernel for nn_Attention (RMSNorm + QKV + RoPE +
causal SDPA + out-proj), Megatron-style head-parallel over 8 NeuronCores.

Strategy per core c (owns heads 2c, 2c+1):
  phase 0: RMSNorm of its own 512-token chunk, in transposed layout
           (features on partitions), via ones-matmul column reduction.
  AG:      AllGather of normalized activations xn^T (bf16, 1MB/rank).
  phase 3: Q/K/V projections for its two heads (bf16 matmuls), RoPE via
           host-precomputed cos/sin tables + partition-swap, V transposed
           on TensorE into [keys, dims] blocks with a ones column appended
           (softmax denominator comes out of the same matmul as context).
  phase 4: causal SDPA in S^T layout: scores^T = K^T_blk.T @ Q^T
           (two heads row-packed on the PE array), exp on ScalarE (no max
           subtraction - scores are O(6) so fp32 exp is safe), diag-block
           masking, AV matmul with ones-augmented V accumulating
           [ctx^T; denom] in PSUM; deferred division by the denominator.
  A2A:     AllToAll redistributes context from head-sharded to
           token-sharded (1MB/rank).
  phase 6: out-projection for its own 512-token chunk only.
Host does layout-only prep (transpose, head-column permutation, constant
RoPE/mask tables) and the final concat.
"""
import sys

sys.path.insert(0, "/opt/trn_rl_repo")

import numpy as np
import ml_dtypes
from contextlib import ExitStack

import concourse.bass as bass
import concourse.mybir as mybir
import concourse.tile as tile
from concourse import bacc
from concourse.bass_utils import run_bass_kernel_spmd
from concourse.masks import make_identity

F32 = mybir.dt.float32
BF16 = mybir.dt.bfloat16

B, S, D, H, DH = 2, 2048, 1024, 16, 64
NC = 8
TOK = B * S            # 4096
CHUNK = TOK // NC      # 512
EPS = 1.1920929e-07
THETA = 10000.0
NKB = S // 128         # key blocks per batch: 16
QT = S // 512          # q tiles per batch: 4

_CACHE = {}
DEBUG = False


def _build():
    nc = bacc.Bacc("TRN2", target_bir_lowering=False, debug=False, num_devices=NC)

    xt_d = nc.dram_tensor("xt", [D, CHUNK], F32, kind="ExternalInput")
    nw_d = nc.dram_tensor("nw", [D, 1], F32, kind="ExternalInput")
    wq_d = nc.dram_tensor("wqc", [D, 128], F32, kind="ExternalInput")
    wk_d = nc.dram_tensor("wkc", [D, 128], F32, kind="ExternalInput")
    wv_d = nc.dram_tensor("wvc", [D, 128], F32, kind="ExternalInput")
    wo_d = nc.dram_tensor("wo", [D, D], F32, kind="ExternalInput")
    cos_d = nc.dram_tensor("cosb", [128, TOK], BF16, kind="ExternalInput")
    sin_d = nc.dram_tensor("sinb", [128, TOK], BF16, kind="ExternalInput")
    msk_d = nc.dram_tensor("dmask", [128, 128], BF16, kind="ExternalInput")
    out_d = nc.dram_tensor("out", [CHUNK, D], F32, kind="ExternalOutput")

    agx_in = nc.dram_tensor("agx_in", [D, CHUNK], BF16)
    agx_out = nc.dram_tensor("agx_out", [NC * D, CHUNK], BF16, addr_space="Shared")
    agr_in = nc.dram_tensor("agr_in", [1, CHUNK], BF16)
    agr_out = nc.dram_tensor("agr_out", [NC, CHUNK], BF16, addr_space="Shared")
    warm_in = nc.dram_tensor("warm_in", [1, 128], BF16)
    warm_out = nc.dram_tensor("warm_out", [NC, 128], BF16, addr_space="Shared")
    a2a_in = nc.dram_tensor("a2a_in", [NC * 128, CHUNK], BF16)
    a2a_out = nc.dram_tensor("a2a_out", [NC * 128, CHUNK], BF16)
    dbg = {}
    if DEBUG:
        for nm, shp in (("dbg_xn0", [128, TOK]), ("dbg_q", [128, TOK]),
                        ("dbg_k", [128, TOK]), ("dbg_v", [128, TOK]),
                        ("dbg_vall", [128, B * NKB * 130]),
                        ("dbg_ctx", [128, TOK]), ("dbg_ctxg", [128, NC * CHUNK])):
            dbg[nm] = nc.dram_tensor(nm, shp, BF16, kind="ExternalOutput")
        for nm in ("dbg_den", "dbg_rec"):
            dbg[nm] = nc.dram_tensor(nm, [16, 512], F32, kind="ExternalOutput")

    with tile.TileContext(nc) as tc, ExitStack() as ctx:
        pp = ctx.enter_context(tc.tile_pool(name="persist", bufs=1))

        # ---- persistent tiles ----
        qT = pp.tile([128, TOK], BF16, tag="qT")
        kT = pp.tile([128, TOK], BF16, tag="kT")
        vT = pp.tile([128, TOK], BF16, tag="vT")
        v_all = pp.tile([128, B * NKB * 130], BF16, tag="v_all")
        cosT = pp.tile([128, TOK], BF16, tag="cosT")
        sinT = pp.tile([128, TOK], BF16, tag="sinT")
        dmaskT = pp.tile([128, 128], BF16, tag="dmaskT")
        identT = pp.tile([128, 128], BF16, tag="identT")
        ones128 = pp.tile([128, 1], BF16, tag="ones128")
        ones1 = pp.tile([1, 128], BF16, tag="ones1")
        nw_sb = pp.tile([128, 8], F32, tag="nw_sb")
        w_sb = {p: pp.tile([128, 8, 128], BF16, name=f"w_{p}", tag=f"w_{p}") for p in "qkv"}
        wo_sb = pp.tile([128, 8, 1024], BF16, tag="wo_sb")
        ctx_sb = pp.tile([128, TOK], BF16, tag="ctx_sb")

        nc.vector.memset(ones128, 1.0)
        nc.vector.memset(ones1, 1.0)
        nc.gpsimd.collective_compute(
            "AllGather", mybir.AluOpType.bypass,
            replica_groups=[list(range(NC))],
            ins=[warm_in.ap().opt()], outs=[warm_out.ap().opt()])

        # ---- phase 0: cast raw x^T chunk to bf16 + own-chunk inv-rms ----
        # The AllGather ships RAW x (bf16) plus a per-chunk 1/rms row; the
        # normalization is applied after the gather by folding 1/rms into the
        # RoPE tables (RoPE commutes with per-token scalars) and one extra
        # multiply for V. norm_w is folded into the weight casts. This gets
        # the collective started ~60us earlier than normalizing first.
        with tc.tile_pool(name="rms", bufs=2) as rms_pool, \
             tc.tile_pool(name="ps0", bufs=1, space="PSUM") as ps0:
            ssq = ps0.tile([1, CHUNK], F32, tag="ssq")
            for kt in range(8):
                xb = rms_pool.tile([128, CHUNK], BF16, tag="xb")
                for h in range(2):
                    r0 = kt * 128 + h * 64
                    xtile = rms_pool.tile([64, CHUNK], F32, tag=f"xt{h}")
                    nc.sync.dma_start(out=xtile, in_=xt_d[r0 : r0 + 64, :])
                    nc.vector.tensor_copy(xb[h * 64 : (h + 1) * 64, :], xtile)
                    nc.sync.dma_start(out=agx_in[r0 : r0 + 64, :], in_=xb[h * 64 : (h + 1) * 64, :])
                xsq = rms_pool.tile([128, CHUNK], BF16, tag="xsq")
                nc.vector.tensor_mul(xsq, xb, xb)
                nc.tensor.matmul(ssq, ones128, xsq, start=(kt == 0), stop=(kt == 7))
            eps_t = rms_pool.tile([1, 1], F32, tag="eps_t")
            nc.vector.memset(eps_t, float(EPS))
            rstd = rms_pool.tile([1, CHUNK], F32, tag="rstd")
            nc.scalar.activation(rstd, ssq, mybir.ActivationFunctionType.Sqrt,
                                 bias=eps_t[0:1, 0:1], scale=1.0 / D)
            inv = rms_pool.tile([1, CHUNK], F32, tag="inv")
            nc.vector.reciprocal_approx_fast(out=inv, in_=rstd)
            invb = rms_pool.tile([1, CHUNK], BF16, tag="invb")
            nc.vector.tensor_copy(invb, inv)
            nc.sync.dma_start(out=agr_in[0:1, :], in_=invb)

        # ---- AllGather raw x^T + inv-rms rows ----
        nc.gpsimd.collective_compute(
            "AllGather", mybir.AluOpType.bypass,
            replica_groups=[list(range(NC))],
            ins=[agx_in.ap().opt()], outs=[agx_out.ap().opt()])
        nc.gpsimd.collective_compute(
            "AllGather", mybir.AluOpType.bypass,
            replica_groups=[list(range(NC))],
            ins=[agr_in.ap().opt()], outs=[agr_out.ap().opt()])

        nc.sync.dma_start(out=cosT, in_=cos_d[:, :])
        nc.sync.dma_start(out=sinT, in_=sin_d[:, :])
        nc.sync.dma_start(out=dmaskT, in_=msk_d[:, :])
        make_identity(nc, identT)
        for kt in range(8):
            nc.sync.dma_start(out=nw_sb[:, kt : kt + 1], in_=nw_d[kt * 128 : (kt + 1) * 128, :])

        # weight staging + cast to bf16
        with tc.tile_pool(name="wstage", bufs=3) as wstage:
            for p, d in (("q", wq_d), ("k", wk_d), ("v", wv_d)):
                for kt in range(8):
                    st = wstage.tile([128, 128], F32, tag="wst")
                    nc.sync.dma_start(out=st, in_=d[kt * 128 : (kt + 1) * 128, :])
                    nc.vector.tensor_scalar_mul(w_sb[p][:, kt, :], st, nw_sb[:, kt : kt + 1])


        with tc.tile_pool(name="xn", bufs=1) as xn_pool, \
             tc.tile_pool(name="tmp", bufs=1) as tmp_pool, \
             tc.tile_pool(name="ps3", bufs=3, space="PSUM") as ps3:
            # gathered raw x^T -> SBUF as per-(ktile, chunk) tiles so each
            # projection matmul only waits for its own 128KB slab
            agx_v = agx_out.ap().rearrange("(c r) t -> r c t", r=D)
            xn_sb = [[None] * NC for _ in range(8)]
            for kt in range(8):
                for c in range(NC):
                    t = xn_pool.tile([128, CHUNK], BF16, name=f"xn{kt}_{c}", tag=f"xn{kt}_{c}")
                    nc.sync.dma_start(out=t, in_=agx_v[kt * 128 : (kt + 1) * 128, c])
                    xn_sb[kt][c] = t
            # inv-rms rows -> [1, 4096], broadcast to rbc [128, 4096]
            invg = xn_pool.tile([1, TOK], BF16, tag="invg")
            nc.sync.dma_start(out=invg.rearrange("p (c t) -> p c t", c=NC),
                              in_=agr_out.ap()[:, :])
            rbc = xn_pool.tile([128, TOK], BF16, tag="rbc")
            for tt in range(8):
                rp = ps3.tile([128, 512], F32, name="rp", tag="rp", bufs=2)
                nc.tensor.matmul(rp, ones1, invg[0:1, tt * 512 : (tt + 1) * 512],
                                 start=True, stop=True)
                nc.scalar.copy(rbc[:, tt * 512 : (tt + 1) * 512], rp)
            # fold 1/rms into the RoPE tables (q, k) ...
            nc.vector.tensor_mul(cosT, cosT, rbc)
            nc.vector.tensor_mul(sinT, sinT, rbc)

            # ---- phase 3: projections + RoPE + V transpose ----
            def project(dest, wkey):
                for tt in range(8):
                    ps = ps3.tile([128, 512], F32, tag="proj")
                    for kt in range(8):
                        nc.tensor.matmul(ps, w_sb[wkey][:, kt, :],
                                         xn_sb[kt][tt],
                                         start=(kt == 0), stop=(kt == 7))
                    nc.scalar.copy(dest[:, tt * 512 : (tt + 1) * 512], ps)

            def rope(t):
                sw = tmp_pool.tile([128, TOK], BF16, tag="sw")
                for a, b2 in ((0, 32), (64, 96)):
                    nc.sync.dma_start(out=sw[a : a + 32, :], in_=t[b2 : b2 + 32, :])
                    nc.sync.dma_start(out=sw[b2 : b2 + 32, :], in_=t[a : a + 32, :])
                nc.vector.tensor_mul(t, t, cosT)
                nc.vector.tensor_mul(sw, sw, sinT)
                nc.vector.tensor_add(t, t, sw)

            if DEBUG:
                nc.sync.dma_start(out=dbg["dbg_xn0"][:, :], in_=xn_sb[0])
            project(qT, "q")
            rope(qT)
            project(kT, "k")
            rope(kT)
            project(vT, "v")
            nc.vector.tensor_mul(vT, vT, rbc)
            vv = v_all.rearrange("p (blk c) -> p blk c", c=130)
            for b in range(B):
                for kb in range(NKB):
                    blk = b * NKB + kb
                    pst = ps3.tile([128, 128], BF16, tag="vtr")
                    nc.tensor.transpose(pst, vT[:, b * S + kb * 128 : b * S + (kb + 1) * 128], identT)
                    nc.scalar.copy(vv[:, blk, 0:64], pst[:, 0:64])
                    nc.scalar.copy(vv[:, blk, 65:129], pst[:, 64:128])
            nc.gpsimd.memset(vv[:, :, 64:65], 1.0)
            nc.gpsimd.memset(vv[:, :, 129:130], 1.0)
            if DEBUG:
                nc.sync.dma_start(out=dbg["dbg_q"][:, :], in_=qT)
                nc.sync.dma_start(out=dbg["dbg_k"][:, :], in_=kT)
                nc.sync.dma_start(out=dbg["dbg_v"][:, :], in_=vT)
                nc.sync.dma_start(out=dbg["dbg_vall"][:, :], in_=v_all)

        # stage wo during SDPA (emitted late so its DMAs don't contend with
        # the front of the kernel)
        with tc.tile_pool(name="wstage2", bufs=2) as wstage2:
            for kt in range(8):
                st = wstage2.tile([128, 1024], F32, tag="wost")
                nc.sync.dma_start(out=st, in_=wo_d[kt * 128 : (kt + 1) * 128, :])
                nc.vector.tensor_copy(wo_sb[:, kt, :], st)

        # ---- phase 4: SDPA ----
        # The two batches are interleaved so one batch's exp (ScalarE) overlaps
        # the other's matmuls (TensorE). Scores share one 3-buf PSUM tag; the
        # four context accumulators (2 batches x 2 heads) get their own banks.
        with tc.tile_pool(name="pexp", bufs=6) as pexp, \
             tc.tile_pool(name="cnorm", bufs=2) as cnorm, \
             tc.tile_pool(name="ps4", bufs=3, space="PSUM") as ps4, \
             tc.tile_pool(name="ps4b", bufs=1, space="PSUM") as ps4b, \
             tc.tile_pool(name="ps4c", bufs=1, space="PSUM") as ps4c:
            for step in range(B * QT):
                b, j = step % B, step // B
                base = b * S
                ctxp = {0: ps4c.tile([65, 512], F32, name=f"ctxA{b}", tag=f"ctxA{b}"),
                        1: ps4c.tile([65, 512], F32, name=f"ctxB{b}", tag=f"ctxB{b}")}
                nkb = 4 * (j + 1)
                for kb in range(nkb):
                    m = kb - 4 * j
                    c0 = 128 * m if m >= 0 else 0
                    w = 512 - c0
                    qcol0 = base + 512 * j + c0
                    koff = base + kb * 128
                    for hi, r0 in ((0, 0), (1, 64)):
                        sc = ps4.tile([128, 512], F32, name="sc", tag="sc")
                        nc.tensor.matmul(
                            sc[:, 0:w],
                            kT[r0 : r0 + 64, koff : koff + 128],
                            qT[r0 : r0 + 64, qcol0 : qcol0 + w],
                            start=True, stop=True)
                        p = pexp.tile([128, 512], BF16, name=f"p{hi}", tag=f"p{hi}")
                        nc.scalar.activation(
                            p[:, c0:512], sc[:, 0:w],
                            mybir.ActivationFunctionType.Exp, scale=0.125)
                        if m >= 0:
                            nc.vector.tensor_mul(p[:, c0 : c0 + 128], p[:, c0 : c0 + 128], dmaskT)
                        vcol = (b * NKB + kb) * 130 + hi * 65
                        nc.tensor.matmul(
                            ctxp[hi][:, c0:512],
                            v_all[:, vcol : vcol + 65],
                            p[:, c0:512],
                            start=(kb == 0), stop=(kb == nkb - 1),
                            skip_group_check=True)
                # normalize: ctx / denom (denom = row 64 of ctx psum)
                for hi, r0 in ((0, 0), (1, 64)):
                    den_s = cnorm.tile([1, 512], F32, tag="den_s")
                    nc.vector.tensor_copy(den_s, ctxp[hi][64:65, :])
                    rec = cnorm.tile([1, 512], F32, tag="rec")
                    nc.vector.reciprocal_approx_fast(out=rec, in_=den_s)
                    if DEBUG:
                        di = ((b * QT + j) * 2 + hi)
                        nc.sync.dma_start(out=dbg["dbg_den"][di : di + 1, :], in_=den_s)
                        nc.sync.dma_start(out=dbg["dbg_rec"][di : di + 1, :], in_=rec)
                    recb = cnorm.tile([1, 512], BF16, tag="recb")
                    nc.vector.tensor_copy(recb, rec)
                    bc = ps4b.tile([64, 512], F32, name="bc", tag="bc")
                    nc.tensor.matmul(bc, ones1[0:1, 0:64], recb, start=True, stop=True)
                    cth = cnorm.tile([64, 512], F32, tag="cth")
                    nc.vector.tensor_copy(cth, ctxp[hi][0:64, :])
                    nc.vector.tensor_mul(
                        ctx_sb[r0 : r0 + 64, base + 512 * j : base + 512 * (j + 1)],
                        cth, bc)

        if DEBUG:
            nc.sync.dma_start(out=dbg["dbg_ctx"][:, :], in_=ctx_sb)
        # ---- AllToAll: head-sharded ctx -> token-sharded ctx ----
        # a2a_in block c' = our ctx rows for chunk c'; received block i =
        # rank i's ctx rows (global rows 128i..128i+127) for OUR chunk.
        for cb in range(NC):
            nc.sync.dma_start(out=a2a_in[cb * 128 : (cb + 1) * 128, :],
                              in_=ctx_sb[:, cb * CHUNK : (cb + 1) * CHUNK])
        nc.gpsimd.collective_compute(
            "AllToAll", mybir.AluOpType.bypass,
            replica_groups=[list(range(NC))],
            ins=[a2a_in.ap().opt()], outs=[a2a_out.ap().opt()])

        # ---- phase 6: out-projection on own token chunk ----
        with tc.tile_pool(name="ctxgp", bufs=1) as ctxgp, \
             tc.tile_pool(name="outp", bufs=3) as outp, \
             tc.tile_pool(name="ps6", bufs=2, space="PSUM") as ps6:
            ctxg = ctxgp.tile([128, NC * CHUNK], BF16, tag="ctxg")
            for cb in range(NC):
                nc.sync.dma_start(out=ctxg[:, cb * CHUNK : (cb + 1) * CHUNK],
                                  in_=a2a_out[cb * 128 : (cb + 1) * 128, :])
            if DEBUG:
                nc.sync.dma_start(out=dbg["dbg_ctxg"][:, :], in_=ctxg)
            for tl in range(4):
                pso = {nh: ps6.tile([128, 512], F32, name=f"op{nh}", tag=f"op{nh}") for nh in range(2)}
                for nh in range(2):
                    for cb in range(8):
                        nc.tensor.matmul(
                            pso[nh],
                            ctxg[:, cb * CHUNK + tl * 128 : cb * CHUNK + (tl + 1) * 128],
                            wo_sb[:, cb, nh * 512 : (nh + 1) * 512],
                            start=(cb == 0), stop=(cb == 7))
                ost = outp.tile([128, 1024], F32, tag="ost")
                nc.scalar.copy(ost[:, 0:512], pso[0])
                nc.scalar.copy(ost[:, 512:1024], pso[1])
                nc.sync.dma_start(out=out_d[tl * 128 : tl * 128 + 64, :], in_=ost[0:64, :])
                nc.sync.dma_start(out=out_d[tl * 128 + 64 : (tl + 1) * 128, :], in_=ost[64:128, :])

    nc.compile()
    return nc


def _head_cols(h, deinterleave):
    base = h * DH
    if deinterleave:
        return np.concatenate([base + np.arange(0, DH, 2), base + np.arange(1, DH, 2)])
    return base + np.arange(DH)


def _make_tables():
    inv_freq = 1.0 / (THETA ** (np.arange(0, DH, 2) / DH))   # [32]
    ang = np.arange(S)[:, None] * inv_freq[None, :]          # [2048, 32]
    ch = np.cos(ang).T.astype(np.float32)                    # [32, 2048]
    sh = np.sin(ang).T.astype(np.float32)
    cosb = np.tile(np.concatenate([ch, ch, ch, ch], axis=0), (1, B))
    sinb = np.tile(np.concatenate([-sh, sh, -sh, sh], axis=0), (1, B))
    kk, qq = np.meshgrid(np.arange(128), np.arange(128), indexing="ij")
    dmask = (kk <= qq).astype(np.float32)
    bf = ml_dtypes.bfloat16
    return cosb.astype(bf), sinb.astype(bf), dmask.astype(bf)


def _in_maps(inputs):
    x = np.ascontiguousarray(inputs["x"], dtype=np.float32)
    norm_w = np.asarray(inputs["norm_w"], dtype=np.float32)
    wq = np.asarray(inputs["wq"], dtype=np.float32)
    wk = np.asarray(inputs["wk"], dtype=np.float32)
    wv = np.asarray(inputs["wv"], dtype=np.float32)
    wo = np.ascontiguousarray(inputs["wo"], dtype=np.float32)

    xT = np.ascontiguousarray(x.reshape(TOK, D).T)           # [1024, 4096]
    cosb, sinb, dmask = _make_tables()
    nw = np.ascontiguousarray(norm_w.reshape(D, 1))

    maps = []
    for c in range(NC):
        hA, hB = 2 * c, 2 * c + 1
        qcols = np.concatenate([_head_cols(hA, True), _head_cols(hB, True)])
        vcols = np.concatenate([_head_cols(hA, False), _head_cols(hB, False)])
        maps.append({
            "xt": np.ascontiguousarray(xT[:, c * CHUNK : (c + 1) * CHUNK]),
            "nw": nw,
            "wqc": np.ascontiguousarray(wq[:, qcols]),
            "wkc": np.ascontiguousarray(wk[:, qcols]),
            "wvc": np.ascontiguousarray(wv[:, vcols]),
            "wo": wo,
            "cosb": cosb,
            "sinb": sinb,
            "dmask": dmask,
        })
    return maps


def _run(inputs, trace=False):
    if "nc" not in _CACHE:
        _CACHE["nc"] = _build()
    nc = _CACHE["nc"]
    res = run_bass_kernel_spmd(nc, _in_maps(inputs), core_ids=list(range(NC)),
                               trace=trace)
    chunks = [res.results[c]["out"] for c in range(NC)]
    out = np.concatenate(chunks, axis=0).reshape(B, S, D).astype(np.float32)
    return out, res


def kernel(**inputs) -> np.ndarray:
    out, _ = _run(inputs, trace=False)
    return out


# revision 22
# speedup vs baseline: 1.0117x; 1.0087x over previous
"""Distributed Trainium2 kernel for nn_Attention (RMSNorm + QKV + RoPE +
causal SDPA + out-proj), Megatron-style head-parallel over 8 NeuronCores.

Strategy per core c (owns heads 2c, 2c+1):
  phase 0: RMSNorm of its own 512-token chunk, in transposed layout
           (features on partitions), via ones-matmul column reduction.
  AG:      AllGather of normalized activations xn^T (bf16, 1MB/rank).
  phase 3: Q/K/V projections for its two heads (bf16 matmuls), RoPE via
           host-precomputed cos/sin tables + partition-swap, V transposed
           on TensorE into [keys, dims] blocks with a ones column appended
           (softmax denominator comes out of the same matmul as context).
  phase 4: causal SDPA in S^T layout: scores^T = K^T_blk.T @ Q^T
           (two heads row-packed on the PE array), exp on ScalarE (no max
           subtraction - scores are O(6) so fp32 exp is safe), diag-block
           masking, AV matmul with ones-augmented V accumulating
           [ctx^T; denom] in PSUM; deferred division by the denominator.
  A2A:     AllToAll redistributes context from head-sharded to
           token-sharded (1MB/rank).
  phase 6: out-projection for its own 512-token chunk only.
Host does layout-only prep (transpose, head-column permutation, constant
RoPE/mask tables) and the final concat.
"""
import sys

sys.path.insert(0, "/opt/trn_rl_repo")

import numpy as np
import ml_dtypes
from contextlib import ExitStack

import concourse.bass as bass
import concourse.mybir as mybir
import concourse.tile as tile
from concourse import bacc
from concourse.bass_utils import run_bass_kernel_spmd
from concourse.masks import make_identity

F32 = mybir.dt.float32
BF16 = mybir.dt.bfloat16

B, S, D, H, DH = 2, 2048, 1024, 16, 64
NC = 8
TOK = B * S            # 4096
CHUNK = TOK // NC      # 512
EPS = 1.1920929e-07
THETA = 10000.0
NKB = S // 128         # key blocks per batch: 16
QT = S // 512          # q tiles per batch: 4

_CACHE = {}
DEBUG = False


def _build():
    nc = bacc.Bacc("TRN2", target_bir_lowering=False, debug=False, num_devices=NC)

    xt_d = nc.dram_tensor("xt", [D, CHUNK], F32, kind="ExternalInput")
    nw_d = nc.dram_tensor("nw", [D, 1], F32, kind="ExternalInput")
    wq_d = nc.dram_tensor("wqc", [D, 128], F32, kind="ExternalInput")
    wk_d = nc.dram_tensor("wkc", [D, 128], F32, kind="ExternalInput")
    wv_d = nc.dram_tensor("wvc", [D, 128], F32, kind="ExternalInput")
    wo_d = nc.dram_tensor("wo", [D, D], F32, kind="ExternalInput")
    cos_d = nc.dram_tensor("cosb", [128, TOK], BF16, kind="ExternalInput")
    sin_d = nc.dram_tensor("sinb", [128, TOK], BF16, kind="ExternalInput")
    msk_d = nc.dram_tensor("dmask", [128, 128], BF16, kind="ExternalInput")
    out_d = nc.dram_tensor("out", [CHUNK, D], F32, kind="ExternalOutput")

    agx_in = nc.dram_tensor("agx_in", [D, CHUNK], BF16)
    agx_out = nc.dram_tensor("agx_out", [NC * D, CHUNK], BF16, addr_space="Shared")
    agr_in = nc.dram_tensor("agr_in", [1, CHUNK], BF16)
    agr_out = nc.dram_tensor("agr_out", [NC, CHUNK], BF16, addr_space="Shared")
    a2a_in = nc.dram_tensor("a2a_in", [NC * 128, CHUNK], BF16)
    a2a_out = nc.dram_tensor("a2a_out", [NC * 128, CHUNK], BF16)
    dbg = {}
    if DEBUG:
        for nm, shp in (("dbg_xn0", [128, TOK]), ("dbg_q", [128, TOK]),
                        ("dbg_k", [128, TOK]), ("dbg_v", [128, TOK]),
                        ("dbg_vall", [128, B * NKB * 130]),
                        ("dbg_ctx", [128, TOK]), ("dbg_ctxg", [128, NC * CHUNK])):
            dbg[nm] = nc.dram_tensor(nm, shp, BF16, kind="ExternalOutput")
        for nm in ("dbg_den", "dbg_rec"):
            dbg[nm] = nc.dram_tensor(nm, [16, 512], F32, kind="ExternalOutput")

    with tile.TileContext(nc) as tc, ExitStack() as ctx:
        pp = ctx.enter_context(tc.tile_pool(name="persist", bufs=1))

        # ---- persistent tiles ----
        qT = pp.tile([128, TOK], BF16, tag="qT")
        kT = pp.tile([128, TOK], BF16, tag="kT")
        vT = pp.tile([128, TOK], BF16, tag="vT")
        v_all = pp.tile([128, B * NKB * 130], BF16, tag="v_all")
        cosT = pp.tile([128, TOK], BF16, tag="cosT")
        sinT = pp.tile([128, TOK], BF16, tag="sinT")
        dmaskT = pp.tile([128, 128], BF16, tag="dmaskT")
        identT = pp.tile([128, 128], BF16, tag="identT")
        ones128 = pp.tile([128, 1], BF16, tag="ones128")
        ones1 = pp.tile([1, 128], BF16, tag="ones1")
        nw_sb = pp.tile([128, 8], F32, tag="nw_sb")
        w_sb = {p: pp.tile([128, 8, 128], BF16, name=f"w_{p}", tag=f"w_{p}") for p in "qkv"}
        wo_sb = pp.tile([128, 8, 1024], BF16, tag="wo_sb")
        ctx_sb = pp.tile([128, TOK], BF16, tag="ctx_sb")

        nc.vector.memset(ones128, 1.0)
        nc.vector.memset(ones1, 1.0)
        # ---- phase 0: cast raw x^T chunk to bf16 + own-chunk inv-rms ----
        # The AllGather ships RAW x (bf16) plus a per-chunk 1/rms row; the
        # normalization is applied after the gather by folding 1/rms into the
        # RoPE tables (RoPE commutes with per-token scalars) and one extra
        # multiply for V. norm_w is folded into the weight casts. This gets
        # the collective started ~60us earlier than normalizing first.
        with tc.tile_pool(name="rms", bufs=2) as rms_pool, \
             tc.tile_pool(name="ps0", bufs=1, space="PSUM") as ps0:
            ssq = ps0.tile([1, CHUNK], F32, tag="ssq")
            for kt in range(8):
                xb = rms_pool.tile([128, CHUNK], BF16, tag="xb")
                for h in range(2):
                    r0 = kt * 128 + h * 64
                    xtile = rms_pool.tile([64, CHUNK], F32, tag=f"xt{h}")
                    nc.sync.dma_start(out=xtile, in_=xt_d[r0 : r0 + 64, :])
                    nc.vector.tensor_copy(xb[h * 64 : (h + 1) * 64, :], xtile)
                    nc.sync.dma_start(out=agx_in[r0 : r0 + 64, :], in_=xb[h * 64 : (h + 1) * 64, :])
                xsq = rms_pool.tile([128, CHUNK], BF16, tag="xsq")
                nc.vector.tensor_mul(xsq, xb, xb)
                nc.tensor.matmul(ssq, ones128, xsq, start=(kt == 0), stop=(kt == 7))
            eps_t = rms_pool.tile([1, 1], F32, tag="eps_t")
            nc.vector.memset(eps_t, float(EPS))
            rstd = rms_pool.tile([1, CHUNK], F32, tag="rstd")
            nc.scalar.activation(rstd, ssq, mybir.ActivationFunctionType.Sqrt,
                                 bias=eps_t[0:1, 0:1], scale=1.0 / D)
            inv = rms_pool.tile([1, CHUNK], F32, tag="inv")
            nc.vector.reciprocal_approx_fast(out=inv, in_=rstd)
            invb = rms_pool.tile([1, CHUNK], BF16, tag="invb")
            nc.vector.tensor_copy(invb, inv)
            nc.sync.dma_start(out=agr_in[0:1, :], in_=invb)

        # ---- AllGather raw x^T + inv-rms rows ----
        nc.gpsimd.collective_compute(
            "AllGather", mybir.AluOpType.bypass,
            replica_groups=[list(range(NC))],
            ins=[agx_in.ap().opt()], outs=[agx_out.ap().opt()])
        nc.gpsimd.collective_compute(
            "AllGather", mybir.AluOpType.bypass,
            replica_groups=[list(range(NC))],
            ins=[agr_in.ap().opt()], outs=[agr_out.ap().opt()])

        nc.sync.dma_start(out=cosT, in_=cos_d[:, :])
        nc.sync.dma_start(out=sinT, in_=sin_d[:, :])
        nc.sync.dma_start(out=dmaskT, in_=msk_d[:, :])
        make_identity(nc, identT)
        for kt in range(8):
            nc.sync.dma_start(out=nw_sb[:, kt : kt + 1], in_=nw_d[kt * 128 : (kt + 1) * 128, :])

        # weight staging + cast to bf16
        with tc.tile_pool(name="wstage", bufs=3) as wstage:
            for p, d in (("q", wq_d), ("k", wk_d), ("v", wv_d)):
                for kt in range(8):
                    st = wstage.tile([128, 128], F32, tag="wst")
                    nc.sync.dma_start(out=st, in_=d[kt * 128 : (kt + 1) * 128, :])
                    nc.vector.tensor_scalar_mul(w_sb[p][:, kt, :], st, nw_sb[:, kt : kt + 1])


        with tc.tile_pool(name="xn", bufs=1) as xn_pool, \
             tc.tile_pool(name="tmp", bufs=1) as tmp_pool, \
             tc.tile_pool(name="ps3", bufs=3, space="PSUM") as ps3:
            # gathered raw x^T -> SBUF as per-(ktile, chunk) tiles so each
            # projection matmul only waits for its own 128KB slab
            agx_v = agx_out.ap().rearrange("(c r) t -> r c t", r=D)
            xn_sb = [[None] * NC for _ in range(8)]
            for kt in range(8):
                for c in range(NC):
                    t = xn_pool.tile([128, CHUNK], BF16, name=f"xn{kt}_{c}", tag=f"xn{kt}_{c}")
                    nc.sync.dma_start(out=t, in_=agx_v[kt * 128 : (kt + 1) * 128, c])
                    xn_sb[kt][c] = t
            # inv-rms rows -> [1, 4096], broadcast to rbc [128, 4096]
            invg = xn_pool.tile([1, TOK], BF16, tag="invg")
            nc.sync.dma_start(out=invg.rearrange("p (c t) -> p c t", c=NC),
                              in_=agr_out.ap()[:, :])
            rbc = xn_pool.tile([128, TOK], BF16, tag="rbc")
            for tt in range(8):
                rp = ps3.tile([128, 512], F32, name="rp", tag="rp", bufs=2)
                nc.tensor.matmul(rp, ones1, invg[0:1, tt * 512 : (tt + 1) * 512],
                                 start=True, stop=True)
                nc.scalar.copy(rbc[:, tt * 512 : (tt + 1) * 512], rp)
            # fold 1/rms into the RoPE tables (q, k) ...
            nc.vector.tensor_mul(cosT, cosT, rbc)
            nc.vector.tensor_mul(sinT, sinT, rbc)

            # ---- phase 3: projections + RoPE + V transpose ----
            def project(dest, wkey):
                for tt in range(8):
                    ps = ps3.tile([128, 512], F32, tag="proj")
                    for kt in range(8):
                        nc.tensor.matmul(ps, w_sb[wkey][:, kt, :],
                                         xn_sb[kt][tt],
                                         start=(kt == 0), stop=(kt == 7))
                    nc.scalar.copy(dest[:, tt * 512 : (tt + 1) * 512], ps)

            def rope(t):
                sw = tmp_pool.tile([128, TOK], BF16, tag="sw")
                for a, b2 in ((0, 32), (64, 96)):
                    nc.sync.dma_start(out=sw[a : a + 32, :], in_=t[b2 : b2 + 32, :])
                    nc.sync.dma_start(out=sw[b2 : b2 + 32, :], in_=t[a : a + 32, :])
                nc.vector.tensor_mul(t, t, cosT)
                nc.vector.tensor_mul(sw, sw, sinT)
                nc.vector.tensor_add(t, t, sw)

            if DEBUG:
                nc.sync.dma_start(out=dbg["dbg_xn0"][:, :], in_=xn_sb[0])
            project(qT, "q")
            rope(qT)
            project(kT, "k")
            rope(kT)
            project(vT, "v")
            nc.vector.tensor_mul(vT, vT, rbc)
            vv = v_all.rearrange("p (blk c) -> p blk c", c=130)
            for b in range(B):
                for kb in range(NKB):
                    blk = b * NKB + kb
                    pst = ps3.tile([128, 128], BF16, tag="vtr")
                    nc.tensor.transpose(pst, vT[:, b * S + kb * 128 : b * S + (kb + 1) * 128], identT)
                    nc.scalar.copy(vv[:, blk, 0:64], pst[:, 0:64])
                    nc.scalar.copy(vv[:, blk, 65:129], pst[:, 64:128])
            nc.gpsimd.memset(vv[:, :, 64:65], 1.0)
            nc.gpsimd.memset(vv[:, :, 129:130], 1.0)
            if DEBUG:
                nc.sync.dma_start(out=dbg["dbg_q"][:, :], in_=qT)
                nc.sync.dma_start(out=dbg["dbg_k"][:, :], in_=kT)
                nc.sync.dma_start(out=dbg["dbg_v"][:, :], in_=vT)
                nc.sync.dma_start(out=dbg["dbg_vall"][:, :], in_=v_all)

        # stage wo during SDPA (emitted late so its DMAs don't contend with
        # the front of the kernel)
        with tc.tile_pool(name="wstage2", bufs=2) as wstage2:
            for kt in range(8):
                st = wstage2.tile([128, 1024], F32, tag="wost")
                nc.sync.dma_start(out=st, in_=wo_d[kt * 128 : (kt + 1) * 128, :])
                nc.vector.tensor_copy(wo_sb[:, kt, :], st)

        # ---- phase 4: SDPA ----
        # The two batches are interleaved so one batch's exp (ScalarE) overlaps
        # the other's matmuls (TensorE). Scores share one 3-buf PSUM tag; the
        # four context accumulators (2 batches x 2 heads) get their own banks.
        with tc.tile_pool(name="pexp", bufs=6) as pexp, \
             tc.tile_pool(name="cnorm", bufs=2) as cnorm, \
             tc.tile_pool(name="ps4", bufs=3, space="PSUM") as ps4, \
             tc.tile_pool(name="ps4b", bufs=1, space="PSUM") as ps4b, \
             tc.tile_pool(name="ps4c", bufs=1, space="PSUM") as ps4c:
            for step in range(B * QT):
                b, j = step % B, step // B
                base = b * S
                ctxp = {0: ps4c.tile([65, 512], F32, name=f"ctxA{b}", tag=f"ctxA{b}"),
                        1: ps4c.tile([65, 512], F32, name=f"ctxB{b}", tag=f"ctxB{b}")}
                nkb = 4 * (j + 1)
                for kb in range(nkb):
                    m = kb - 4 * j
                    c0 = 128 * m if m >= 0 else 0
                    w = 512 - c0
                    qcol0 = base + 512 * j + c0
                    koff = base + kb * 128
                    for hi, r0 in ((0, 0), (1, 64)):
                        sc = ps4.tile([128, 512], F32, name="sc", tag="sc")
                        nc.tensor.matmul(
                            sc[:, 0:w],
                            kT[r0 : r0 + 64, koff : koff + 128],
                            qT[r0 : r0 + 64, qcol0 : qcol0 + w],
                            start=True, stop=True)
                        p = pexp.tile([128, 512], BF16, name=f"p{hi}", tag=f"p{hi}")
                        nc.scalar.activation(
                            p[:, c0:512], sc[:, 0:w],
                            mybir.ActivationFunctionType.Exp, scale=0.125)
                        if m >= 0:
                            nc.vector.tensor_mul(p[:, c0 : c0 + 128], p[:, c0 : c0 + 128], dmaskT)
                        vcol = (b * NKB + kb) * 130 + hi * 65
                        nc.tensor.matmul(
                            ctxp[hi][:, c0:512],
                            v_all[:, vcol : vcol + 65],
                            p[:, c0:512],
                            start=(kb == 0), stop=(kb == nkb - 1),
                            skip_group_check=True)
                # normalize: ctx / denom (denom = row 64 of ctx psum)
                for hi, r0 in ((0, 0), (1, 64)):
                    den_s = cnorm.tile([1, 512], F32, tag="den_s")
                    nc.vector.tensor_copy(den_s, ctxp[hi][64:65, :])
                    rec = cnorm.tile([1, 512], F32, tag="rec")
                    nc.vector.reciprocal_approx_fast(out=rec, in_=den_s)
                    if DEBUG:
                        di = ((b * QT + j) * 2 + hi)
                        nc.sync.dma_start(out=dbg["dbg_den"][di : di + 1, :], in_=den_s)
                        nc.sync.dma_start(out=dbg["dbg_rec"][di : di + 1, :], in_=rec)
                    recb = cnorm.tile([1, 512], BF16, tag="recb")
                    nc.vector.tensor_copy(recb, rec)
                    bc = ps4b.tile([64, 512], F32, name="bc", tag="bc")
                    nc.tensor.matmul(bc, ones1[0:1, 0:64], recb, start=True, stop=True)
                    cth = cnorm.tile([64, 512], F32, tag="cth")
                    nc.vector.tensor_copy(cth, ctxp[hi][0:64, :])
                    nc.vector.tensor_mul(
                        ctx_sb[r0 : r0 + 64, base + 512 * j : base + 512 * (j + 1)],
                        cth, bc)
                cch = b * QT + j
                nc.sync.dma_start(
                    out=a2a_in[cch * 128 : (cch + 1) * 128, :],
                    in_=ctx_sb[:, cch * CHUNK : (cch + 1) * CHUNK])

        if DEBUG:
            nc.sync.dma_start(out=dbg["dbg_ctx"][:, :], in_=ctx_sb)
        # ---- AllToAll: head-sharded ctx -> token-sharded ctx ----
        # a2a_in block c' = our ctx rows for chunk c'; received block i =
        # rank i's ctx rows (global rows 128i..128i+127) for OUR chunk.
        nc.gpsimd.collective_compute(
            "AllToAll", mybir.AluOpType.bypass,
            replica_groups=[list(range(NC))],
            ins=[a2a_in.ap().opt()], outs=[a2a_out.ap().opt()])

        # ---- phase 6: out-projection on own token chunk ----
        with tc.tile_pool(name="ctxgp", bufs=1) as ctxgp, \
             tc.tile_pool(name="outp", bufs=3) as outp, \
             tc.tile_pool(name="ps6", bufs=2, space="PSUM") as ps6:
            ctxg = ctxgp.tile([128, NC * CHUNK], BF16, tag="ctxg")
            for cb in range(NC):
                nc.sync.dma_start(out=ctxg[:, cb * CHUNK : (cb + 1) * CHUNK],
                                  in_=a2a_out[cb * 128 : (cb + 1) * 128, :])
            if DEBUG:
                nc.sync.dma_start(out=dbg["dbg_ctxg"][:, :], in_=ctxg)
            for tl in range(4):
                pso = {nh: ps6.tile([128, 512], F32, name=f"op{nh}", tag=f"op{nh}") for nh in range(2)}
                for nh in range(2):
                    for cb in range(8):
                        nc.tensor.matmul(
                            pso[nh],
                            ctxg[:, cb * CHUNK + tl * 128 : cb * CHUNK + (tl + 1) * 128],
                            wo_sb[:, cb, nh * 512 : (nh + 1) * 512],
                            start=(cb == 0), stop=(cb == 7))
                ost = outp.tile([128, 1024], F32, tag="ost")
                nc.scalar.copy(ost[:, 0:512], pso[0])
                nc.scalar.copy(ost[:, 512:1024], pso[1])
                nc.sync.dma_start(out=out_d[tl * 128 : tl * 128 + 64, :], in_=ost[0:64, :])
                nc.sync.dma_start(out=out_d[tl * 128 + 64 : (tl + 1) * 128, :], in_=ost[64:128, :])

    nc.compile()
    return nc


def _head_cols(h, deinterleave):
    base = h * DH
    if deinterleave:
        return np.concatenate([base + np.arange(0, DH, 2), base + np.arange(1, DH, 2)])
    return base + np.arange(DH)


def _make_tables():
    inv_freq = 1.0 / (THETA ** (np.arange(0, DH, 2) / DH))   # [32]
    ang = np.arange(S)[:, None] * inv_freq[None, :]          # [2048, 32]
    ch = np.cos(ang).T.astype(np.float32)                    # [32, 2048]
    sh = np.sin(ang).T.astype(np.float32)
    cosb = np.tile(np.concatenate([ch, ch, ch, ch], axis=0), (1, B))
    sinb = np.tile(np.concatenate([-sh, sh, -sh, sh], axis=0), (1, B))
    kk, qq = np.meshgrid(np.arange(128), np.arange(128), indexing="ij")
    dmask = (kk <= qq).astype(np.float32)
    bf = ml_dtypes.bfloat16
    return cosb.astype(bf), sinb.astype(bf), dmask.astype(bf)


def _in_maps(inputs):
    x = np.ascontiguousarray(inputs["x"], dtype=np.float32)
    norm_w = np.asarray(inputs["norm_w"], dtype=np.float32)
    wq = np.asarray(inputs["wq"], dtype=np.float32)
    wk = np.asarray(inputs["wk"], dtype=np.float32)
    wv = np.asarray(inputs["wv"], dtype=np.float32)
    wo = np.ascontiguousarray(inputs["wo"], dtype=np.float32)

    xT = np.ascontiguousarray(x.reshape(TOK, D).T)           # [1024, 4096]
    cosb, sinb, dmask = _make_tables()
    nw = np.ascontiguousarray(norm_w.reshape(D, 1))

    maps = []
    for c in range(NC):
        hA, hB = 2 * c, 2 * c + 1
        qcols = np.concatenate([_head_cols(hA, True), _head_cols(hB, True)])
        vcols = np.concatenate([_head_cols(hA, False), _head_cols(hB, False)])
        maps.append({
            "xt": np.ascontiguousarray(xT[:, c * CHUNK : (c + 1) * CHUNK]),
            "nw": nw,
            "wqc": np.ascontiguousarray(wq[:, qcols]),
            "wkc": np.ascontiguousarray(wk[:, qcols]),
            "wvc": np.ascontiguousarray(wv[:, vcols]),
            "wo": wo,
            "cosb": cosb,
            "sinb": sinb,
            "dmask": dmask,
        })
    return maps


def _run(inputs, trace=False):
    if "nc" not in _CACHE:
        _CACHE["nc"] = _build()
    nc = _CACHE["nc"]
    res = run_bass_kernel_spmd(nc, _in_maps(inputs), core_ids=list(range(NC)),
                               trace=trace)
    chunks = [res.results[c]["out"] for c in range(NC)]
    out = np.concatenate(chunks, axis=0).reshape(B, S, D).astype(np.float32)
    return out, res


def kernel(**inputs) -> np.ndarray:
    out, _ = _run(inputs, trace=False)
    return out
